# revision 15
# baseline (speedup 1.0000x reference)
"""Trainium2 Bass kernel for GQA causal sliding-window self-attention.

Problem: B=2, T=2048, C=1024, H=16 query heads, KV=4 kv heads, D=64,
window=1024, with value-embedding gating and RoPE+RMS on q/k.

Sharding: sequence-parallel with halo. 8 cores = batch(2) x T-chunks(4x512).
Each core receives a padded 1536-row context slice of x/ve (its 512 query rows
plus the 1024 preceding rows, zero-padded below row 0), computes k/v for the
whole context and q for its own rows, runs banded attention, projects through
Wo and writes its [512, 1024] output chunk. No collectives.

Geometry is uniform across cores: each q-tile (128 rows) attends exactly 9
context s-tiles; the first is masked with a strict-lower-triangular pattern
(window edge), the last with a causal pattern (diagonal), and zero-padded
context rows are masked via an additive mask row carried as a 65th contraction
row in the scores matmul (k-transpose carries the mask vector, q-transpose
carries a row of ones).
"""
import sys

for p in ("/opt/trn_rl_repo", "/root/.axon_site/_ro/trn_rl_repo"):
    if p not in sys.path:
        sys.path.insert(0, p)

import numpy as np

import concourse.bass as bass
import concourse.bacc as bacc
import concourse.mybir as mybir
import concourse.tile as tile
from concourse import bass_utils
from concourse.masks import make_identity

F32 = mybir.dt.float32
BF16 = mybir.dt.bfloat16
AF = mybir.ActivationFunctionType
ALU = mybir.AluOpType
AX = mybir.AxisListType

B, T, C = 2, 2048, 1024
H, KV, D = 16, 4, 64
GROUPS = H // KV          # 4 query heads per kv head
WINDOW = 1024
CHUNK = 512               # query rows per core
CTX = CHUNK + WINDOW      # 1536 context rows per core
N_CORES = 8
TT = CTX // 128           # 12 context t-tiles
QT = CHUNK // 128         # 4 query tiles
CB = C // 128             # 8 contraction blocks
NST = WINDOW // 128 + 1   # 9 s-tiles per q-tile
EPS = 1.1920929e-07
NEG = -30000.0            # additive mask (scores are |.| <= ~80; exp(NEG/8)=0)

_CACHED = {}


def _bcast_free(ap, n, keep_free_dims):
    """Insert a step-0 broadcast dim of size n before the trailing free dims."""
    return bass.AP(
        tensor=ap.tensor,
        offset=ap.offset,
        ap=[ap.ap[0], [0, n]] + list(ap.ap[1:]),
    )


def build_program():
    nc = bacc.Bacc("TRN2", num_devices=N_CORES, debug=False)

    x_d = nc.dram_tensor("x_s", [C, CTX], F32, kind="ExternalInput").ap()  # pre-transposed on host
    ve_d = nc.dram_tensor("ve_s", [CTX, KV * D], F32, kind="ExternalInput").ap()
    cos_d = nc.dram_tensor("cos_s", [CTX, D // 2], F32, kind="ExternalInput").ap()
    sin_d = nc.dram_tensor("sin_s", [CTX, D // 2], F32, kind="ExternalInput").ap()
    msk_d = nc.dram_tensor("mask_s", [1, CTX], F32, kind="ExternalInput").ap()
    wq_d = nc.dram_tensor("Wq", [C, H * D], F32, kind="ExternalInput").ap()
    wk_d = nc.dram_tensor("Wk", [C, KV * D], F32, kind="ExternalInput").ap()
    wv_d = nc.dram_tensor("Wv", [C, KV * D], F32, kind="ExternalInput").ap()
    wo_d = nc.dram_tensor("Wo", [C, C], F32, kind="ExternalInput").ap()
    wg_d = nc.dram_tensor("Wg", [32, KV], F32, kind="ExternalInput").ap()
    out_d = nc.dram_tensor("out", [CHUNK, C], F32, kind="ExternalOutput").ap()

    with tile.TileContext(nc) as tc:
        _emit(tc, x_d, ve_d, cos_d, sin_d, msk_d, wq_d, wk_d, wv_d, wo_d, wg_d, out_d)
    nc.compile()
    return nc


def _emit(tc, x_d, ve_d, cos_d, sin_d, msk_d, wq_d, wk_d, wv_d, wo_d, wg_d, out_d):
    nc = tc.nc
    from contextlib import ExitStack

    ctx = ExitStack()
    with ctx:
        consts = ctx.enter_context(tc.tile_pool(name="consts", bufs=1))
        wpool = ctx.enter_context(tc.tile_pool(name="weights", bufs=1))
        xtp = ctx.enter_context(tc.tile_pool(name="xT", bufs=1))
        kvwork = ctx.enter_context(tc.tile_pool(name="kvwork", bufs=2))
        kvout = ctx.enter_context(tc.tile_pool(name="kvout", bufs=1))
        qwork = ctx.enter_context(tc.tile_pool(name="qwork", bufs=1))
        qout = ctx.enter_context(tc.tile_pool(name="qout", bufs=1))
        stats = ctx.enter_context(tc.tile_pool(name="stats", bufs=1))
        probs_p = ctx.enter_context(tc.tile_pool(name="probs", bufs=2))
        ytp = ctx.enter_context(tc.tile_pool(name="yT", bufs=1))
        reps = ctx.enter_context(tc.tile_pool(name="reps", bufs=2))
        outp = ctx.enter_context(tc.tile_pool(name="outsb", bufs=2))

        dram_p = ctx.enter_context(tc.tile_pool(name="dram", bufs=2, space="DRAM"))

        # ---------------- constants & weights ----------------
        ident = consts.tile([128, 128], BF16, tag="ident")
        make_identity(nc, ident[:])
        # P_edge: valid where s' > t'  -> NEG where s' <= t'
        p_edge = consts.tile([128, 128], BF16, tag="p_edge")
        nc.gpsimd.memset(p_edge[:], 0.0)
        nc.gpsimd.affine_select(
            out=p_edge[:], in_=p_edge[:], compare_op=ALU.is_gt, fill=NEG,
            base=0, pattern=[[-1, 128]], channel_multiplier=1,
        )
        # P_diag: valid where s' <= t' -> NEG where s' > t'
        p_diag = consts.tile([128, 128], BF16, tag="p_diag")
        nc.gpsimd.memset(p_diag[:], 0.0)
        nc.gpsimd.affine_select(
            out=p_diag[:], in_=p_diag[:], compare_op=ALU.is_ge, fill=NEG,
            base=0, pattern=[[1, 128]], channel_multiplier=-1,
        )
        ones_col = consts.tile([128, 1], BF16, tag="ones")
        nc.vector.memset(ones_col[:], 1.0)
        eps_sb = consts.tile([128, 1], F32, tag="eps")
        nc.vector.memset(eps_sb[:], EPS)
        maskrow = consts.tile([1, CTX], BF16, tag="maskrow")
        nc.gpsimd.dma_start(out=maskrow[:], in_=msk_d[:])

        cos_sb = consts.tile([128, TT, 32], F32, tag="cos")
        sin_sb = consts.tile([128, TT, 32], F32, tag="sin")
        nc.sync.dma_start(out=cos_sb[:], in_=cos_d.rearrange("(k p) d -> p k d", p=128))
        nc.sync.dma_start(out=sin_sb[:], in_=sin_d.rearrange("(k p) d -> p k d", p=128))

        wq = []
        wkv = []
        wo = []
        for cb in range(CB):
            t = wpool.tile([128, H * D], BF16, tag=f"wq{cb}")
            nc.gpsimd.dma_start(out=t[:], in_=wq_d[cb * 128:(cb + 1) * 128, :])
            wq.append(t)
            t = wpool.tile([128, 2 * KV * D], BF16, tag=f"wkv{cb}")
            nc.gpsimd.dma_start(out=t[:, 0:256], in_=wk_d[cb * 128:(cb + 1) * 128, :])
            nc.gpsimd.dma_start(out=t[:, 256:512], in_=wv_d[cb * 128:(cb + 1) * 128, :])
            wkv.append(t)
            t = wpool.tile([128, C], BF16, tag=f"wo{cb}")
            nc.gpsimd.dma_start(out=t[:], in_=wo_d[cb * 128:(cb + 1) * 128, :])
            wo.append(t)
        wg = consts.tile([32, KV], BF16, tag="wg")
        nc.gpsimd.dma_start(out=wg[:], in_=wg_d[:])

        # ---------------- xT: cast-DMA from host-pre-transposed x ----------------
        xT = [xtp.tile([128, CTX], BF16, tag=f"xT{cb}", name=f"xT{cb}") for cb in range(CB)]
        for cb in range(CB):
            nc.gpsimd.dma_start(out=xT[cb][:], in_=x_d[cb * 128:(cb + 1) * 128, :])

        # ---------------- kv projection + gating + rope/rms ----------------
        ps_proj_cm = tc.tile_pool(name="ps_proj", bufs=2, space="PSUM")
        ps_proj = ps_proj_cm.__enter__()
        kn_bf = kvout.tile([128, TT, KV * D], BF16, tag="kn")
        # v with a ones column appended per kv head: [.., g*65:g*65+64] = v_g,
        # [.., g*65+64] = 1.0 (gives softmax denominators as PV output row 64)
        v_b = kvout.tile([128, TT, KV * 65], BF16, tag="vb")
        ones_cols = bass.AP(
            tensor=v_b.tensor, offset=v_b.offset + 64,
            ap=[v_b.ap[0], [KV * 65, TT], [65, KV]],
        )
        nc.vector.memset(ones_cols, 1.0)
        ssk = stats.tile([128, TT, KV], F32, tag="ssk")

        for kt in range(TT):
            kv_ps = ps_proj.tile([128, 512], F32, tag="kv_ps")
            for cb in range(CB):
                nc.tensor.matmul(
                    kv_ps[:], xT[cb][:, kt * 128:(kt + 1) * 128], wkv[cb][:],
                    start=(cb == 0), stop=(cb == CB - 1),
                )
            gate_ps = ps_proj.tile([128, KV], F32, tag="gate_ps")
            nc.tensor.matmul(
                gate_ps[:], xT[0][0:32, kt * 128:(kt + 1) * 128], wg[:],
                start=True, stop=True,
            )
            # gate2 = tanh(u/2); v_new = v + ve*(1+gate2)
            g2 = kvwork.tile([128, KV], F32, tag="g2")
            nc.scalar.activation(out=g2[:], in_=gate_ps[:], func=AF.Tanh, scale=0.5)
            ve_sb = kvwork.tile([128, KV * D], F32, tag="ve")
            nc.sync.dma_start(out=ve_sb[:], in_=ve_d[kt * 128:(kt + 1) * 128, :])
            veg = kvwork.tile([128, KV, D], F32, tag="veg")
            g2b = bass.AP(tensor=g2.tensor, offset=g2.offset,
                          ap=[g2.ap[0], [g2.ap[1][0], KV], [0, D]])
            nc.vector.tensor_tensor(
                out=veg[:], in0=ve_sb[:].rearrange("p (g d) -> p g d", g=KV),
                in1=g2b, op=ALU.mult,
            )
            s1 = kvwork.tile([128, KV * D], F32, tag="s1")
            nc.vector.tensor_tensor(
                out=s1[:], in0=kv_ps[:, 256:512],
                in1=veg[:].rearrange("p g d -> p (g d)"), op=ALU.add,
            )
            v_dst = bass.AP(
                tensor=v_b.tensor, offset=v_b.offset + kt * (KV * 65),
                ap=[v_b.ap[0], [65, KV], [1, D]],
            )
            nc.vector.tensor_tensor(
                out=v_dst, in0=s1[:].rearrange("p (g d) -> p g d", g=KV),
                in1=ve_sb[:].rearrange("p (g d) -> p g d", g=KV), op=ALU.add,
            )
            # k: square for rms (pre-rope, rope preserves row sums of squares)
            sqk = kvwork.tile([128, KV * D], F32, tag="rotk", bufs=1)
            nc.scalar.activation(out=sqk[:], in_=kv_ps[:, 0:256], func=AF.Square)
            nc.vector.tensor_reduce(
                out=ssk[:, kt, :], in_=sqk[:].rearrange("p (g d) -> p g d", g=KV),
                axis=AX.X, op=ALU.add,
            )
            # rope on k (from psum)
            k3 = kv_ps[:, 0:256].rearrange("p (g d) -> p g d", g=KV)
            cosb = _bcast_free(cos_sb[:, kt, :], KV, 1)
            sinb = _bcast_free(sin_sb[:, kt, :], KV, 1)
            rot = kvwork.tile([128, KV, D], F32, tag="rotk", bufs=1)
            ta = kvwork.tile([128, KV, 32], F32, tag="ka")
            tb = kvwork.tile([128, KV, 32], F32, tag="kb")
            nc.vector.tensor_tensor(out=ta[:], in0=k3[:, :, 0:32], in1=cosb, op=ALU.mult)
            nc.vector.tensor_tensor(out=tb[:], in0=k3[:, :, 32:64], in1=sinb, op=ALU.mult)
            nc.vector.tensor_tensor(out=rot[:, :, 0:32], in0=ta[:], in1=tb[:], op=ALU.add)
            nc.vector.tensor_tensor(out=ta[:], in0=k3[:, :, 32:64], in1=cosb, op=ALU.mult)
            nc.vector.tensor_tensor(out=tb[:], in0=k3[:, :, 0:32], in1=sinb, op=ALU.mult)
            nc.vector.tensor_tensor(out=rot[:, :, 32:64], in0=ta[:], in1=tb[:], op=ALU.subtract)
            # normalize: inv = exp(-0.5*ln(ss/64 + eps)); kn = rot * inv
            lnk = kvwork.tile([128, KV], F32, tag="lnk")
            nc.scalar.activation(out=lnk[:], in_=ssk[:, kt, :], func=AF.Ln,
                                 scale=1.0 / D, bias=eps_sb[:])
            invk = kvwork.tile([128, KV], F32, tag="invk")
            nc.scalar.activation(out=invk[:], in_=lnk[:], func=AF.Exp, scale=-0.5)
            invb = bass.AP(tensor=invk.tensor, offset=invk.offset,
                           ap=[invk.ap[0], [invk.ap[1][0], KV], [0, D]])
            nc.vector.tensor_tensor(
                out=kn_bf[:, kt, :].rearrange("p (g d) -> p g d", g=KV),
                in0=rot[:], in1=invb, op=ALU.mult,
            )

        # transpose kn -> kTm per group, with mask row appended
        kTm = [kvout.tile([65, CTX], BF16, tag=f"kTm{g}", name=f"kTm{g}") for g in range(KV)]
        for gp in range(2):  # group pairs (0,1) and (2,3)
            kTp = kvwork.tile([128, CTX], BF16, tag="kTp", bufs=1)
            for kt in range(TT):
                nc.sync.dma_start(
                    out=kTp[:, kt * 128:(kt + 1) * 128],
                    in_=kn_bf[:, kt, gp * 128:(gp + 1) * 128],
                    transpose=True,
                )
            nc.sync.dma_start(out=kTm[2 * gp][0:64, :], in_=kTp[0:64, :])
            nc.sync.dma_start(out=kTm[2 * gp + 1][0:64, :], in_=kTp[64:128, :])
        for g in range(KV):
            nc.sync.dma_start(out=kTm[g][64:65, :], in_=maskrow[:])

        # ---------------- q projection + rope/rms + transpose ----------------
        qT1 = [qout.tile([65, CHUNK], BF16, tag=f"qT1_{h}", name=f"qT1_{h}") for h in range(H)]
        for h in range(H):
            nc.vector.memset(qT1[h][64:65, :], 1.0)

        for qt in range(QT):
            kt = 8 + qt  # ctx tile holding this q tile's rows
            q_ps = ps_proj.tile([128, H * D], F32, tag="q_ps")
            for half in range(2):
                for cb in range(CB):
                    nc.tensor.matmul(
                        q_ps[:, half * 512:(half + 1) * 512],
                        xT[cb][:, kt * 128:(kt + 1) * 128],
                        wq[cb][:, half * 512:(half + 1) * 512],
                        start=(cb == 0), stop=(cb == CB - 1),
                    )
            sqq = qwork.tile([128, H * D], F32, tag="rotq")
            nc.scalar.activation(out=sqq[:], in_=q_ps[:], func=AF.Square)
            ssq = stats.tile([128, H], F32, tag=f"ssq{qt}")
            nc.vector.tensor_reduce(
                out=ssq[:], in_=sqq[:].rearrange("p (h d) -> p h d", h=H),
                axis=AX.X, op=ALU.add,
            )
            q3 = q_ps[:].rearrange("p (h d) -> p h d", h=H)
            cosb = _bcast_free(cos_sb[:, kt, :], H, 1)
            sinb = _bcast_free(sin_sb[:, kt, :], H, 1)
            rot = qwork.tile([128, H, D], F32, tag="rotq")
            ta = qwork.tile([128, H, 32], F32, tag="qa")
            tb = qwork.tile([128, H, 32], F32, tag="qb")
            nc.vector.tensor_tensor(out=ta[:], in0=q3[:, :, 0:32], in1=cosb, op=ALU.mult)
            nc.vector.tensor_tensor(out=tb[:], in0=q3[:, :, 32:64], in1=sinb, op=ALU.mult)
            nc.vector.tensor_tensor(out=rot[:, :, 0:32], in0=ta[:], in1=tb[:], op=ALU.add)
            nc.vector.tensor_tensor(out=ta[:], in0=q3[:, :, 32:64], in1=cosb, op=ALU.mult)
            nc.vector.tensor_tensor(out=tb[:], in0=q3[:, :, 0:32], in1=sinb, op=ALU.mult)
            nc.vector.tensor_tensor(out=rot[:, :, 32:64], in0=ta[:], in1=tb[:], op=ALU.subtract)
            lnq = qwork.tile([128, H], F32, tag="lnq")
            nc.scalar.activation(out=lnq[:], in_=ssq[:], func=AF.Ln,
                                 scale=1.0 / D, bias=eps_sb[:])
            invq = qwork.tile([128, H], F32, tag="invq")
            nc.scalar.activation(out=invq[:], in_=lnq[:], func=AF.Exp, scale=-0.5)
            invb = bass.AP(tensor=invq.tensor, offset=invq.offset,
                           ap=[invq.ap[0], [invq.ap[1][0], H], [0, D]])
            qn = qwork.tile([128, H * D], BF16, tag="qn")
            nc.vector.tensor_tensor(
                out=qn[:].rearrange("p (h d) -> p h d", h=H),
                in0=rot[:], in1=invb, op=ALU.mult,
            )
            for hp in range(H // 2):
                qTp = qwork.tile([128, 128], BF16, tag="qTp")
                nc.sync.dma_start(
                    out=qTp[:], in_=qn[:, hp * 128:(hp + 1) * 128], transpose=True,
                )
                nc.sync.dma_start(
                    out=qT1[2 * hp][0:64, qt * 128:(qt + 1) * 128], in_=qTp[0:64, :])
                nc.sync.dma_start(
                    out=qT1[2 * hp + 1][0:64, qt * 128:(qt + 1) * 128], in_=qTp[64:128, :])

        # ---------------- attention ----------------
        ps_proj_cm.__exit__(None, None, None)
        ps_score_cm = tc.tile_pool(name="ps_score", bufs=2, space="PSUM")
        ps_score = ps_score_cm.__enter__()
        ps_pv_cm = tc.tile_pool(name="ps_pv", bufs=1, space="PSUM")
        ps_pv = ps_pv_cm.__enter__()
        yT = [ytp.tile([128, CHUNK], BF16, tag=f"yT{eb}", name=f"yT{eb}") for eb in range(CB)]
        for g in range(KV):
            for qth in range(2):
                probs2 = []
                for qq in range(2):
                    qt = 2 * qth + qq
                    probs = probs_p.tile([128, GROUPS, NST * 128], BF16, tag="probs")
                    for h4 in range(GROUPS):
                        h = 4 * g + h4
                        sc = ps_score.tile([128, NST * 128], F32, tag="score")
                        for st in range(NST):
                            ktile = qt + st
                            edge = st in (0, NST - 1)
                            nc.tensor.matmul(
                                sc[:, st * 128:(st + 1) * 128],
                                kTm[g][:, ktile * 128:(ktile + 1) * 128],
                                qT1[h][:, qt * 128:(qt + 1) * 128],
                                start=True, stop=not edge,
                            )
                            if edge:
                                nc.tensor.matmul(
                                    sc[:, st * 128:(st + 1) * 128], ident[:],
                                    p_edge[:] if st == 0 else p_diag[:],
                                    start=False, stop=True,
                                )
                        nc.scalar.activation(
                            out=probs[:, h4, :], in_=sc[:], func=AF.Exp, scale=0.125,
                        )
                    probs2.append(probs)
                # PV with ones-column stationary: out rows 0-63 = v.T @ p,
                # row 64 = softmax denominator. Two heads share each psum bank
                # (sequential groups), h parity selects the 256-col half.
                pv01 = ps_pv.tile([65, 512], F32, tag="pv01")
                pv23 = ps_pv.tile([65, 512], F32, tag="pv23")
                for h4 in range(GROUPS):
                    pvt = pv01 if h4 < 2 else pv23
                    off = 256 * (h4 % 2)
                    for qq in range(2):
                        qt = 2 * qth + qq
                        for st in range(NST):
                            ktile = qt + st
                            nc.tensor.matmul(
                                pvt[:, off + qq * 128: off + (qq + 1) * 128],
                                v_b[:, ktile, g * 65:g * 65 + 65],
                                probs2[qq][:, h4, st * 128:(st + 1) * 128],
                                start=(st == 0), stop=(st == NST - 1),
                            )
                # divide by denominators (row 64), DRAM-roundtrip broadcast
                rec01 = reps.tile([1, 512], F32, tag="rec01")
                rec23 = reps.tile([1, 512], F32, tag="rec23")
                nc.vector.reciprocal(out=rec01[:], in_=pv01[64:65, :])
                nc.vector.reciprocal(out=rec23[:], in_=pv23[64:65, :])
                dsc = dram_p.tile([2, 512], F32, tag="dsc")
                nc.sync.dma_start(out=dsc[0:1, :], in_=rec01[:])
                nc.sync.dma_start(out=dsc[1:2, :], in_=rec23[:])
                for h4 in range(GROUPS):
                    pvt = pv01 if h4 < 2 else pv23
                    off = 256 * (h4 % 2)
                    rep = reps.tile([64, 256], F32, tag="rep")
                    src_b = bass.AP(
                        tensor=dsc.tensor,
                        offset=dsc.offset + (h4 // 2) * 512 + (h4 % 2) * 256,
                        ap=[[0, 64], [1, 256]],
                    )
                    nc.sync.dma_start(out=rep[:], in_=src_b)
                    eb = 2 * g + h4 // 2
                    ro = 64 * (h4 % 2)
                    nc.vector.tensor_tensor(
                        out=yT[eb][ro:ro + 64, qth * 256:(qth + 1) * 256],
                        in0=pvt[0:64, off:off + 256], in1=rep[:], op=ALU.mult,
                    )

        # ---------------- output projection ----------------
        ps_pv_cm.__exit__(None, None, None)
        ps_score_cm.__exit__(None, None, None)
        ps_out_cm = tc.tile_pool(name="ps_out", bufs=2, space="PSUM")
        ps_out = ps_out_cm.__enter__()
        for qt in range(QT):
            out_ps = ps_out.tile([128, C], F32, tag="out_ps")
            for half in range(2):
                for eb in range(CB):
                    nc.tensor.matmul(
                        out_ps[:, half * 512:(half + 1) * 512],
                        yT[eb][:, qt * 128:(qt + 1) * 128],
                        wo[eb][:, half * 512:(half + 1) * 512],
                        start=(eb == 0), stop=(eb == CB - 1),
                    )
            osb = outp.tile([128, C], F32, tag="osb")
            nc.scalar.copy(out=osb[:, 0:512], in_=out_ps[:, 0:512])
            nc.vector.tensor_copy(out=osb[:, 512:1024], in_=out_ps[:, 512:1024])
            nc.sync.dma_start(out=out_d[qt * 128:(qt + 1) * 128, :], in_=osb[:])
        ps_out_cm.__exit__(None, None, None)


def _make_in_maps(x, ve, cos, sin):
    cos2 = np.ascontiguousarray(cos.reshape(T, D // 2))
    sin2 = np.ascontiguousarray(sin.reshape(T, D // 2))
    in_maps = []
    for core in range(N_CORES):
        b, chunk = divmod(core, 4)
        t0 = chunk * CHUNK
        lo = t0 - WINDOW
        pad = max(0, -lo)
        xs = np.zeros((CTX, C), np.float32)
        vs = np.zeros((CTX, KV * D), np.float32)
        xs[pad:] = x[b, lo + pad:t0 + CHUNK]
        vs[pad:] = ve[b, lo + pad:t0 + CHUNK]
        xs = np.ascontiguousarray(xs.T)
        idx = np.clip(np.arange(lo, t0 + CHUNK), 0, T - 1)
        cs = np.ascontiguousarray(cos2[idx])
        sn = np.ascontiguousarray(sin2[idx])
        mv = np.zeros((1, CTX), np.float32)
        mv[0, :pad] = NEG
        in_maps.append({
            "x_s": xs, "ve_s": vs, "cos_s": cs, "sin_s": sn, "mask_s": mv,
        })
    return in_maps


def kernel(x, ve, cos, sin, Wq, Wk, Wv, Wo, Wg, window):
    assert int(window) == WINDOW
    x = np.asarray(x, np.float32)
    ve = np.asarray(ve, np.float32)
    cos = np.asarray(cos, np.float32)
    sin = np.asarray(sin, np.float32)
    if "nc" not in _CACHED:
        _CACHED["nc"] = build_program()
    nc = _CACHED["nc"]
    in_maps = _make_in_maps(x, ve, cos, sin)
    shared = {
        "Wq": np.asarray(Wq, np.float32), "Wk": np.asarray(Wk, np.float32),
        "Wv": np.asarray(Wv, np.float32), "Wo": np.asarray(Wo, np.float32),
        "Wg": np.asarray(Wg, np.float32),
    }
    for m in in_maps:
        m.update(shared)
    res = bass_utils.run_bass_kernel_spmd(
        nc, in_maps, core_ids=list(range(N_CORES)),
    )
    y = np.empty((B, T, C), np.float32)
    for core in range(N_CORES):
        b, chunk = divmod(core, 4)
        y[b, chunk * CHUNK:(chunk + 1) * CHUNK] = res.results[core]["out"]
    return y


# revision 29
# speedup vs baseline: 385.5333x; 385.5333x over previous
"""Trainium2 Bass kernel for GQA causal sliding-window self-attention.

Problem: B=2, T=2048, C=1024, H=16 query heads, KV=4 kv heads, D=64,
window=1024, with value-embedding gating and RoPE+RMS on q/k.

Sharding: sequence-parallel with halo. 8 cores = batch(2) x T-chunks(4x512).
Each core receives a padded 1536-row context slice of x/ve (its 512 query rows
plus the 1024 preceding rows, zero-padded below row 0), computes k/v for the
whole context and q for its own rows, runs banded attention, projects through
Wo and writes its [512, 1024] output chunk. No collectives.

Geometry is uniform across cores: each q-tile (128 rows) attends exactly 9
context s-tiles; the first is masked with a strict-lower-triangular pattern
(window edge), the last with a causal pattern (diagonal), and zero-padded
context rows are masked via an additive mask row carried as a 65th contraction
row in the scores matmul (k-transpose carries the mask vector, q-transpose
carries a row of ones).
"""
import sys

for p in ("/opt/trn_rl_repo", "/root/.axon_site/_ro/trn_rl_repo"):
    if p not in sys.path:
        sys.path.insert(0, p)

import numpy as np

import concourse.bass as bass
import concourse.bacc as bacc
import concourse.mybir as mybir
import concourse.tile as tile
from concourse import bass_utils
from concourse import hw_specs as _hw_specs
from concourse.masks import make_identity

# All ScalarE functions used here (Exp, Ln, Copy, Identity) live in the
# "natural_log_exp_and_others" table set. Blank the other sets (indices
# preserved for walrus id mapping) so the table-load pass emits ONE load
# instead of thrashing between per-anchor sets (~1.3us per load).
_orig_get_act_tables = _hw_specs.get_activation_tables


def _single_set_tables(arch):
    tabs = _orig_get_act_tables(arch)
    return {k: (v if k == "natural_log_exp_and_others" else set())
            for k, v in tabs.items()}


_hw_specs.get_activation_tables = _single_set_tables
try:
    import concourse.bacc as _bacc_mod
    _bacc_mod.get_activation_tables = _single_set_tables
except (ImportError, AttributeError):
    pass

F32 = mybir.dt.float32
BF16 = mybir.dt.bfloat16
AF = mybir.ActivationFunctionType
ALU = mybir.AluOpType
AX = mybir.AxisListType

B, T, C = 2, 2048, 1024
H, KV, D = 16, 4, 64
GROUPS = H // KV          # 4 query heads per kv head
WINDOW = 1024
CHUNK = 512               # query rows per core
CTX = CHUNK + WINDOW      # 1536 context rows per core
N_CORES = 8
TT = CTX // 128           # 12 context t-tiles
QT = CHUNK // 128         # 4 query tiles
CB = C // 128             # 8 contraction blocks
NST = WINDOW // 128 + 1   # 9 s-tiles per q-tile
EPS = 1.1920929e-07
NEG = -30000.0            # additive mask (scores are |.| <= ~80; exp(NEG/8)=0)

_CACHED = {}


def _bcast_free(ap, n, keep_free_dims):
    """Insert a step-0 broadcast dim of size n before the trailing free dims."""
    return bass.AP(
        tensor=ap.tensor,
        offset=ap.offset,
        ap=[ap.ap[0], [0, n]] + list(ap.ap[1:]),
    )


def build_program():
    nc = bacc.Bacc("TRN2", num_devices=N_CORES, debug=False)

    x_d = nc.dram_tensor("x_s", [C, CTX], F32, kind="ExternalInput").ap()  # pre-transposed on host
    ve_d = nc.dram_tensor("ve_s", [CTX, KV * D], F32, kind="ExternalInput").ap()
    cos_d = nc.dram_tensor("cos_s", [CTX, D], F32, kind="ExternalInput").ap()
    sin_d = nc.dram_tensor("sin_s", [CTX, D], F32, kind="ExternalInput").ap()
    msk_d = nc.dram_tensor("mask_s", [1, CTX], F32, kind="ExternalInput").ap()
    wq_d = nc.dram_tensor("Wq", [C, H * D], F32, kind="ExternalInput").ap()
    wk_d = nc.dram_tensor("Wk", [C, KV * D], F32, kind="ExternalInput").ap()
    wv_d = nc.dram_tensor("Wv", [C, KV * D], F32, kind="ExternalInput").ap()
    wo_d = nc.dram_tensor("Wo", [C, C], F32, kind="ExternalInput").ap()
    wg_d = nc.dram_tensor("Wg", [32, KV], F32, kind="ExternalInput").ap()
    out_d = nc.dram_tensor("out", [CHUNK, C], F32, kind="ExternalOutput").ap()

    with tile.TileContext(nc) as tc:
        _emit(tc, x_d, ve_d, cos_d, sin_d, msk_d, wq_d, wk_d, wv_d, wo_d, wg_d, out_d)
    nc.compile()
    return nc


def _emit(tc, x_d, ve_d, cos_d, sin_d, msk_d, wq_d, wk_d, wv_d, wo_d, wg_d, out_d):
    nc = tc.nc
    from contextlib import ExitStack

    ctx = ExitStack()
    with ctx:
        consts = ctx.enter_context(tc.tile_pool(name="consts", bufs=1))
        wpool = ctx.enter_context(tc.tile_pool(name="weights", bufs=1))
        xtp = ctx.enter_context(tc.tile_pool(name="xT", bufs=1))
        kvwork = ctx.enter_context(tc.tile_pool(name="kvwork", bufs=2))
        kvout = ctx.enter_context(tc.tile_pool(name="kvout", bufs=1))
        qwork = ctx.enter_context(tc.tile_pool(name="qwork", bufs=1))
        qout = ctx.enter_context(tc.tile_pool(name="qout", bufs=1))
        stats = ctx.enter_context(tc.tile_pool(name="stats", bufs=1))
        probs_p = ctx.enter_context(tc.tile_pool(name="probs", bufs=2))
        ytp = ctx.enter_context(tc.tile_pool(name="yT", bufs=1))
        reps = ctx.enter_context(tc.tile_pool(name="reps", bufs=2))
        outp = ctx.enter_context(tc.tile_pool(name="outsb", bufs=2))

        dram_p = ctx.enter_context(tc.tile_pool(name="dram", bufs=2, space="DRAM"))

        # ---------------- constants & weights ----------------
        ident = consts.tile([128, 128], BF16, tag="ident")
        make_identity(nc, ident[:])
        # P_edge: valid where s' > t'  -> NEG where s' <= t'
        p_edge = consts.tile([128, 128], BF16, tag="p_edge")
        nc.gpsimd.memset(p_edge[:], 0.0)
        nc.gpsimd.affine_select(
            out=p_edge[:], in_=p_edge[:], compare_op=ALU.is_gt, fill=NEG,
            base=0, pattern=[[-1, 128]], channel_multiplier=1,
        )
        # P_diag: valid where s' <= t' -> NEG where s' > t'
        p_diag = consts.tile([128, 128], BF16, tag="p_diag")
        nc.gpsimd.memset(p_diag[:], 0.0)
        nc.gpsimd.affine_select(
            out=p_diag[:], in_=p_diag[:], compare_op=ALU.is_ge, fill=NEG,
            base=0, pattern=[[1, 128]], channel_multiplier=-1,
        )
        ones_col = consts.tile([128, 1], BF16, tag="ones")
        nc.vector.memset(ones_col[:], 1.0)
        eps_sb = consts.tile([128, 1], F32, tag="eps")
        nc.vector.memset(eps_sb[:], EPS)
        maskrow = consts.tile([1, CTX], BF16, tag="maskrow")
        nc.gpsimd.dma_start(out=maskrow[:], in_=msk_d[:])

        cos_sb = consts.tile([128, TT, D], F32, tag="cos")
        sin_sb = consts.tile([128, TT, D], F32, tag="sin")
        nc.sync.dma_start(out=cos_sb[:], in_=cos_d.rearrange("(k p) d -> p k d", p=128))
        nc.sync.dma_start(out=sin_sb[:], in_=sin_d.rearrange("(k p) d -> p k d", p=128))

        # xT first (kv projections need it immediately), then Wk/Wv, then Wq/Wo
        xT = [xtp.tile([128, CTX], BF16, tag=f"xT{cb}", name=f"xT{cb}") for cb in range(CB)]
        wq = []
        wkv = []
        wo = []
        for cb in range(CB):
            nc.gpsimd.dma_start(out=xT[cb][:], in_=x_d[cb * 128:(cb + 1) * 128, :])
            t = wpool.tile([128, 2 * KV * D], BF16, tag=f"wkv{cb}")
            nc.gpsimd.dma_start(out=t[:, 0:256], in_=wk_d[cb * 128:(cb + 1) * 128, :])
            nc.gpsimd.dma_start(out=t[:, 256:512], in_=wv_d[cb * 128:(cb + 1) * 128, :])
            wkv.append(t)
        wg = consts.tile([32, KV], BF16, tag="wg")
        nc.gpsimd.dma_start(out=wg[:], in_=wg_d[:])
        for cb in range(CB):
            t = wpool.tile([128, H * D], BF16, tag=f"wq{cb}")
            nc.gpsimd.dma_start(out=t[:], in_=wq_d[cb * 128:(cb + 1) * 128, :])
            wq.append(t)
        for cb in range(CB):
            t = wpool.tile([128, C], BF16, tag=f"wo{cb}")
            nc.gpsimd.dma_start(out=t[:], in_=wo_d[cb * 128:(cb + 1) * 128, :])
            wo.append(t)
        # ---------------- kv projection + gating + rope/rms ----------------
        ps_proj_cm = tc.tile_pool(name="ps_proj", bufs=2, space="PSUM")
        ps_proj = ps_proj_cm.__enter__()
        kn_bf = kvout.tile([128, TT, KV * D], BF16, tag="kn")
        # v with a ones column appended per kv head: [.., g*65:g*65+64] = v_g,
        # [.., g*65+64] = 1.0 (gives softmax denominators as PV output row 64)
        v_b = kvout.tile([128, TT, KV * 65], BF16, tag="vb")
        ones_cols = bass.AP(
            tensor=v_b.tensor, offset=v_b.offset + 64,
            ap=[v_b.ap[0], [KV * 65, TT], [65, KV]],
        )
        nc.gpsimd.memset(ones_cols, 1.0)
        ssk = stats.tile([128, TT, KV], F32, tag="ssk")

        for kt in range(TT):
            kv_ps = ps_proj.tile([128, 512], F32, tag="kv_ps", bufs=3)
            for cb in range(CB):
                nc.tensor.matmul(
                    kv_ps[:], xT[cb][:, kt * 128:(kt + 1) * 128], wkv[cb][:],
                    start=(cb == 0), stop=(cb == CB - 1),
                )
            gate_ps = ps_proj.tile([128, KV], F32, tag="gate_ps", bufs=1)
            nc.tensor.matmul(
                gate_ps[:], xT[0][0:32, kt * 128:(kt + 1) * 128], wg[:],
                start=True, stop=True,
            )
            # gate = 2*sigmoid(u): r = 1/(1+exp(-u)); v_new = v + ve*2r
            gexp = kvwork.tile([128, KV], F32, tag="gexp")
            nc.scalar.activation(out=gexp[:], in_=gate_ps[:], func=AF.Exp, scale=-1.0)
            nc.vector.tensor_scalar_add(gexp[:], gexp[:], 1.0)
            grec = kvwork.tile([128, KV], F32, tag="grec")
            nc.vector.reciprocal(out=grec[:], in_=gexp[:])
            ve_sb = kvwork.tile([128, KV * D], F32, tag="ve")
            nc.sync.dma_start(out=ve_sb[:], in_=ve_d[kt * 128:(kt + 1) * 128, :])
            veg = kvwork.tile([128, KV, D], F32, tag="veg")
            g2b = bass.AP(tensor=grec.tensor, offset=grec.offset,
                          ap=[grec.ap[0], [grec.ap[1][0], KV], [0, D]])
            nc.gpsimd.tensor_tensor(
                out=veg[:], in0=ve_sb[:].rearrange("p (g d) -> p g d", g=KV),
                in1=g2b, op=ALU.mult,
            )
            s1 = kvwork.tile([128, KV, D], F32, tag="s1")
            nc.vector.tensor_tensor(
                out=s1[:], in0=kv_ps[:, 256:512].rearrange("p (g d) -> p g d", g=KV),
                in1=veg[:], op=ALU.add,
            )
            v_dst = bass.AP(
                tensor=v_b.tensor, offset=v_b.offset + kt * (KV * 65),
                ap=[v_b.ap[0], [65, KV], [1, D]],
            )
            nc.gpsimd.tensor_tensor(out=v_dst, in0=s1[:], in1=veg[:], op=ALU.add)
            # rope on k (3 ops, host-expanded tables), then rms from rot
            k3 = kv_ps[:, 0:256].rearrange("p (g d) -> p g d", g=KV)
            ksw = bass.AP(
                tensor=kv_ps.tensor, offset=kv_ps.offset + 32,
                ap=[kv_ps.ap[0], [D, KV], [-32, 2], [1, 32]],
            )
            cosb = _bcast_free(cos_sb[:, kt, :], KV, 1)
            sinb = _bcast_free(sin_sb[:, kt, :], KV, 1)
            kc = kvwork.tile([128, KV, D], F32, tag="kc")
            ks = kvwork.tile([128, KV, 2, 32], F32, tag="ks")
            sinb4 = bass.AP(tensor=sin_sb.tensor,
                            offset=sin_sb.offset + kt * D,
                            ap=[sin_sb.ap[0], [0, KV], [32, 2], [1, 32]])
            nc.vector.tensor_tensor(out=kc[:], in0=k3, in1=cosb, op=ALU.mult)
            nc.vector.tensor_tensor(out=ks[:], in0=ksw, in1=sinb4, op=ALU.mult)
            rot = kvwork.tile([128, KV, D], F32, tag="rotk", bufs=2)
            nc.vector.tensor_tensor(
                out=rot[:], in0=kc[:],
                in1=ks[:].rearrange("p g a d -> p g (a d)"), op=ALU.add)
            sqk = kvwork.tile([128, KV, D], F32, tag="kc")
            nc.vector.tensor_tensor(out=sqk[:], in0=rot[:], in1=rot[:], op=ALU.mult)
            nc.vector.tensor_reduce(
                out=ssk[:, kt, :], in_=sqk[:], axis=AX.X, op=ALU.add,
            )
            lnk = kvwork.tile([128, KV], F32, tag="lnk")
            nc.scalar.activation(out=lnk[:], in_=ssk[:, kt, :], func=AF.Ln,
                                 scale=1.0 / D, bias=eps_sb[:])
            invk = kvwork.tile([128, KV], F32, tag="invk")
            nc.scalar.activation(out=invk[:], in_=lnk[:], func=AF.Exp, scale=-0.5)
            invb = bass.AP(tensor=invk.tensor, offset=invk.offset,
                           ap=[invk.ap[0], [invk.ap[1][0], KV], [0, D]])
            nc.vector.tensor_tensor(
                out=kn_bf[:, kt, :].rearrange("p (g d) -> p g d", g=KV),
                in0=rot[:], in1=invb, op=ALU.mult,
            )

        # transpose kn -> kTm per group, with mask row appended
        kTm = [kvout.tile([65, CTX], BF16, tag=f"kTm{g}", name=f"kTm{g}") for g in range(KV)]
        for gp in range(2):  # group pairs (0,1) and (2,3)
            kTp = kvwork.tile([128, CTX], BF16, tag="kTp", bufs=1)
            for kt in range(TT):
                nc.sync.dma_start(
                    out=kTp[:, kt * 128:(kt + 1) * 128],
                    in_=kn_bf[:, kt, gp * 128:(gp + 1) * 128],
                    transpose=True,
                )
            nc.sync.dma_start(out=kTm[2 * gp][0:64, :], in_=kTp[0:64, :])
            nc.sync.dma_start(out=kTm[2 * gp + 1][0:64, :], in_=kTp[64:128, :])
        for g in range(KV):
            nc.sync.dma_start(out=kTm[g][64:65, :], in_=maskrow[:])

        # ---------------- q projection + rope/rms + transpose ----------------
        qT1 = [qout.tile([65, CHUNK], BF16, tag=f"qT1_{h}", name=f"qT1_{h}") for h in range(H)]
        for h in range(H):
            nc.gpsimd.memset(qT1[h][64:65, :], 1.0)

        qn_all = qout.tile([128, QT, H * D], BF16, tag="qn_all")
        for qhalf in range(2):
            for qq in range(2):
                qt = 2 * qhalf + qq
                kt = 8 + qt  # ctx tile holding this q tile's rows
                q_ps = ps_proj.tile([128, H * D], F32, tag="q_ps")
                for half in range(2):
                    for cb in range(CB):
                        nc.tensor.matmul(
                            q_ps[:, half * 512:(half + 1) * 512],
                            xT[cb][:, kt * 128:(kt + 1) * 128],
                            wq[cb][:, half * 512:(half + 1) * 512],
                            start=(cb == 0), stop=(cb == CB - 1),
                        )
                q3 = q_ps[:].rearrange("p (h d) -> p h d", h=H)
                qsw = bass.AP(
                    tensor=q_ps.tensor, offset=q_ps.offset + 32,
                    ap=[q_ps.ap[0], [D, H], [-32, 2], [1, 32]],
                )
                cosb = _bcast_free(cos_sb[:, kt, :], H, 1)
                sinb4 = bass.AP(tensor=sin_sb.tensor,
                                offset=sin_sb.offset + kt * D,
                                ap=[sin_sb.ap[0], [0, H], [32, 2], [1, 32]])
                qc = qwork.tile([128, H, D], F32, tag="qc")
                qs = qwork.tile([128, H, 2, 32], F32, tag="qs")
                nc.vector.tensor_tensor(out=qc[:], in0=q3, in1=cosb, op=ALU.mult)
                nc.vector.tensor_tensor(out=qs[:], in0=qsw, in1=sinb4, op=ALU.mult)
                rot = qwork.tile([128, H, D], F32, tag="rotq", bufs=2)
                nc.vector.tensor_tensor(
                    out=rot[:], in0=qc[:],
                    in1=qs[:].rearrange("p h a d -> p h (a d)"), op=ALU.add)
                sqq = qwork.tile([128, H, D], F32, tag="qc")
                nc.vector.tensor_tensor(out=sqq[:], in0=rot[:], in1=rot[:], op=ALU.mult)
                ssq = qwork.tile([128, H], F32, tag="ssq")
                nc.vector.tensor_reduce(out=ssq[:], in_=sqq[:], axis=AX.X, op=ALU.add)
                lnq = qwork.tile([128, H], F32, tag="lnq")
                nc.scalar.activation(out=lnq[:], in_=ssq[:], func=AF.Ln,
                                     scale=1.0 / D, bias=eps_sb[:])
                invq = qwork.tile([128, H], F32, tag="invq")
                nc.scalar.activation(out=invq[:], in_=lnq[:], func=AF.Exp, scale=-0.5)
                invb = bass.AP(tensor=invq.tensor, offset=invq.offset,
                               ap=[invq.ap[0], [invq.ap[1][0], H], [0, D]])
                nc.vector.tensor_tensor(
                    out=qn_all[:, qt, :].rearrange("p (h d) -> p h d", h=H),
                    in0=rot[:], in1=invb, op=ALU.mult,
                )
            for hp in range(H // 2):
                qTp = qwork.tile([128, 256], BF16, tag="qTp")
                for qq in range(2):
                    qt = 2 * qhalf + qq
                    nc.sync.dma_start(
                        out=qTp[:, qq * 128:(qq + 1) * 128],
                        in_=qn_all[:, qt, hp * 128:(hp + 1) * 128], transpose=True,
                    )
                sl = slice(qhalf * 256, (qhalf + 1) * 256)
                nc.sync.dma_start(out=qT1[2 * hp][0:64, sl], in_=qTp[0:64, :])
                nc.sync.dma_start(out=qT1[2 * hp + 1][0:64, sl], in_=qTp[64:128, :])

        # ---------------- attention ----------------
        ps_proj_cm.__exit__(None, None, None)
        ps_score_cm = tc.tile_pool(name="ps_score", bufs=2, space="PSUM")
        ps_score = ps_score_cm.__enter__()
        ps_pv_cm = tc.tile_pool(name="ps_pv", bufs=1, space="PSUM")
        ps_pv = ps_pv_cm.__enter__()
        yT = [ytp.tile([128, CHUNK], BF16, tag=f"yT{eb}", name=f"yT{eb}") for eb in range(CB)]
        for g in range(KV):
            for qth in range(2):
                probs2 = []
                for qq in range(2):
                    qt = 2 * qth + qq
                    probs = probs_p.tile([128, GROUPS, NST * 128], BF16, tag="probs")
                    for h4 in range(GROUPS):
                        h = 4 * g + h4
                        sc = ps_score.tile([128, NST * 128], F32, tag="score")
                        for st in range(NST):
                            ktile = qt + st
                            edge = st in (0, NST - 1)
                            nc.tensor.matmul(
                                sc[:, st * 128:(st + 1) * 128],
                                kTm[g][:, ktile * 128:(ktile + 1) * 128],
                                qT1[h][:, qt * 128:(qt + 1) * 128],
                                start=True, stop=not edge,
                            )
                            if edge:
                                nc.tensor.matmul(
                                    sc[:, st * 128:(st + 1) * 128], ident[:],
                                    p_edge[:] if st == 0 else p_diag[:],
                                    start=False, stop=True,
                                )
                        nc.scalar.activation(
                            out=probs[:, h4, :], in_=sc[:], func=AF.Exp, scale=0.125,
                        )
                    probs2.append(probs)
                # PV with ones-column stationary: out rows 0-63 = v.T @ p,
                # row 64 = softmax denominator. Two heads share each psum bank
                # (sequential groups), h parity selects the 256-col half.
                pv01 = ps_pv.tile([65, 512], F32, tag="pv01")
                pv23 = ps_pv.tile([65, 512], F32, tag="pv23")
                for h4 in range(GROUPS):
                    pvt = pv01 if h4 < 2 else pv23
                    off = 256 * (h4 % 2)
                    for qq in range(2):
                        qt = 2 * qth + qq
                        for st in range(NST):
                            ktile = qt + st
                            nc.tensor.matmul(
                                pvt[:, off + qq * 128: off + (qq + 1) * 128],
                                v_b[:, ktile, g * 65:g * 65 + 65],
                                probs2[qq][:, h4, st * 128:(st + 1) * 128],
                                start=(st == 0), stop=(st == NST - 1),
                            )
                # divide by denominators (row 64), DRAM-roundtrip broadcast
                rec01 = reps.tile([1, 512], F32, tag="rec01")
                rec23 = reps.tile([1, 512], F32, tag="rec23")
                nc.vector.reciprocal(out=rec01[:], in_=pv01[64:65, :])
                nc.vector.reciprocal(out=rec23[:], in_=pv23[64:65, :])
                dsc = dram_p.tile([2, 512], F32, tag="dsc")
                nc.sync.dma_start(out=dsc[0:1, :], in_=rec01[:])
                nc.sync.dma_start(out=dsc[1:2, :], in_=rec23[:])
                for pr in range(2):  # head pairs within the group
                    rep = reps.tile([128, 256], F32, tag="rep")
                    src_b = bass.AP(
                        tensor=dsc.tensor, offset=dsc.offset + pr * 512,
                        ap=[[256, 2], [0, 64], [1, 256]],
                    )
                    nc.sync.dma_start(out=rep[:], in_=src_b)
                    eb = 2 * g + pr
                    pvt = pv01 if pr == 0 else pv23
                    nc.vector.tensor_tensor(
                        out=yT[eb][0:64, qth * 256:(qth + 1) * 256],
                        in0=pvt[0:64, 0:256], in1=rep[0:64, :], op=ALU.mult,
                    )
                    nc.vector.tensor_tensor(
                        out=yT[eb][64:128, qth * 256:(qth + 1) * 256],
                        in0=pvt[0:64, 256:512], in1=rep[64:128, :], op=ALU.mult,
                    )

        # ---------------- output projection ----------------
        ps_pv_cm.__exit__(None, None, None)
        ps_score_cm.__exit__(None, None, None)
        ps_out_cm = tc.tile_pool(name="ps_out", bufs=2, space="PSUM")
        ps_out = ps_out_cm.__enter__()
        for qt in range(QT):
            out_ps = ps_out.tile([128, C], F32, tag="out_ps")
            for half in range(2):
                for eb in range(CB):
                    nc.tensor.matmul(
                        out_ps[:, half * 512:(half + 1) * 512],
                        yT[eb][:, qt * 128:(qt + 1) * 128],
                        wo[eb][:, half * 512:(half + 1) * 512],
                        start=(eb == 0), stop=(eb == CB - 1),
                    )
            osb = outp.tile([128, C], F32, tag="osb")
            nc.scalar.copy(out=osb[:, 0:512], in_=out_ps[:, 0:512])
            nc.vector.tensor_copy(out=osb[:, 512:1024], in_=out_ps[:, 512:1024])
            nc.sync.dma_start(out=out_d[qt * 128:(qt + 1) * 128, :], in_=osb[:])
        ps_out_cm.__exit__(None, None, None)


def _make_in_maps(x, ve, cos, sin):
    cos2 = np.ascontiguousarray(cos.reshape(T, D // 2))
    sin2 = np.ascontiguousarray(sin.reshape(T, D // 2))
    in_maps = []
    for core in range(N_CORES):
        b, chunk = divmod(core, 4)
        t0 = chunk * CHUNK
        lo = t0 - WINDOW
        pad = max(0, -lo)
        xs = np.zeros((CTX, C), np.float32)
        vs = np.zeros((CTX, KV * D), np.float32)
        xs[pad:] = x[b, lo + pad:t0 + CHUNK]
        vs[pad:] = ve[b, lo + pad:t0 + CHUNK]
        xs = np.ascontiguousarray(xs.T)
        idx = np.clip(np.arange(lo, t0 + CHUNK), 0, T - 1)
        c_half = cos2[idx]
        s_half = sin2[idx]
        cs = np.ascontiguousarray(np.concatenate([c_half, c_half], axis=1))
        sn = np.ascontiguousarray(np.concatenate([s_half, -s_half], axis=1))
        mv = np.zeros((1, CTX), np.float32)
        mv[0, :pad] = NEG
        in_maps.append({
            "x_s": xs, "ve_s": vs, "cos_s": cs, "sin_s": sn, "mask_s": mv,
        })
    return in_maps


def kernel(x, ve, cos, sin, Wq, Wk, Wv, Wo, Wg, window):
    assert int(window) == WINDOW
    x = np.asarray(x, np.float32)
    ve = np.asarray(ve, np.float32)
    cos = np.asarray(cos, np.float32)
    sin = np.asarray(sin, np.float32)
    if "nc" not in _CACHED:
        _CACHED["nc"] = build_program()
    nc = _CACHED["nc"]
    in_maps = _make_in_maps(x, ve, cos, sin)
    shared = {
        "Wq": np.asarray(Wq, np.float32), "Wk": np.asarray(Wk, np.float32),
        "Wv": np.asarray(Wv, np.float32), "Wo": np.asarray(Wo, np.float32),
        "Wg": np.asarray(Wg, np.float32),
    }
    for m in in_maps:
        m.update(shared)
    res = bass_utils.run_bass_kernel_spmd(
        nc, in_maps, core_ids=list(range(N_CORES)),
    )
    y = np.empty((B, T, C), np.float32)
    for core in range(N_CORES):
        b, chunk = divmod(core, 4)
        y[b, chunk * CHUNK:(chunk + 1) * CHUNK] = res.results[core]["out"]
    return y
